# revision 3
# baseline (speedup 1.0000x reference)
"""Trainium2 Bass kernel for batched weighted scatter-add (AttentionCopy).

Computes out[b, o, v] = sum_i attn[b, o, i] * (ids[b, i] == v)
for ids [16, 512] int32 in [0, 50000), attn [16, 32, 512] f32,
out [16, 32, 50000] f32.

Strategy: pure data parallel over the batch dim — 2 batches per core on 8
cores. Per batch the output is built *densely* on-device via a two-level
one-hot factorization of the vocab index (v = g*1024 + lo, 49 groups):

  out[o, g, lo] = sum_i (attnT[i, o] * (hi_i == g)) * (lo_i == lo)
                = (G_g).T @ Alo        (PE matmul, contraction over i)

The masked matrix G.T[i, (g, o)] and the small one-hot Alo[i, lo] are built
with iota-constant compares on the vector engine; 13 PSUM tiles of
[128, 1024] f32 per batch land in exactly the row-major [32, 50000] layout,
so every DRAM write is a large, fully coalesced DMA.
"""

import sys

sys.path.insert(0, "/opt/trn_rl_repo")

import numpy as np

NCORES = 8
B, O, I = 16, 32, 512
SIZE = 50000
BPC = B // NCORES  # batches per core
V2 = 1024  # lo range (2 PSUM banks)
V1 = 49  # number of vocab groups: 49 * 1024 = 50176 >= 50000
SHIFT = 10
MASK = V2 - 1
PAIRS = V1 * O  # 1568 (g, o) pairs per batch
NT_FULL = PAIRS // 128  # 12 full 128-pair tiles
LAST_W = PAIRS - NT_FULL * 128  # 32 pairs in the final tile (group 48)
LAST_VALID = SIZE - (V1 - 1) * V2  # 848 valid lo columns in group 48
NCHUNK = I // 128  # 4 contraction chunks

_cache = {}


def _build():
    import concourse.bacc as bacc
    import concourse.mybir as mybir
    import concourse.tile as tile

    f32 = mybir.dt.float32
    f16 = mybir.dt.float16
    i32 = mybir.dt.int32
    Alu = mybir.AluOpType

    nc = bacc.Bacc("TRN2", target_bir_lowering=False, debug=False, num_devices=NCORES)

    ids_d = nc.dram_tensor("ids", [BPC, I], i32, kind="ExternalInput").ap()
    attn_d = nc.dram_tensor("attn", [BPC, O, I], f32, kind="ExternalInput").ap()
    gidx_d = nc.dram_tensor("gidx", [128, PAIRS], f16, kind="ExternalInput").ap()
    lov_d = nc.dram_tensor("lov", [128, V2], f16, kind="ExternalInput").ap()
    iden_d = nc.dram_tensor("iden", [128, 128], f32, kind="ExternalInput").ap()
    out_d = nc.dram_tensor("out", [BPC, O, SIZE], f32, kind="ExternalOutput").ap()

    with tile.TileContext(nc) as tc:
        with (
            tc.tile_pool(name="const", bufs=1) as constp,
            tc.tile_pool(name="inp", bufs=2) as inp,
            tc.tile_pool(name="idx", bufs=2) as idxp,
            tc.tile_pool(name="gt", bufs=2) as gtp,
            tc.tile_pool(name="outs", bufs=4) as outp,
            tc.tile_pool(name="psT", bufs=2, space="PSUM") as psT,
            tc.tile_pool(name="psmm", bufs=3, space="PSUM") as psmm,
        ):
            gidx = constp.tile([128, PAIRS], f16)
            nc.sync.dma_start(out=gidx[:], in_=gidx_d[:])
            lov = constp.tile([128, V2], f16)
            nc.sync.dma_start(out=lov[:], in_=lov_d[:])
            iden = constp.tile([128, 128], f32)
            nc.sync.dma_start(out=iden[:], in_=iden_d[:])

            for b in range(BPC):
                ids_t = idxp.tile([128, NCHUNK], i32, tag="ids")
                nc.sync.dma_start(
                    out=ids_t[:], in_=ids_d[b].rearrange("(c p) -> p c", p=128)
                )
                attn_s = inp.tile([O, I], f32, tag="attn")
                nc.sync.dma_start(out=attn_s[:], in_=attn_d[b])

                hi_i = idxp.tile([128, NCHUNK], i32, tag="hi")
                nc.vector.tensor_scalar(
                    out=hi_i[:], in0=ids_t[:], scalar1=SHIFT, scalar2=None,
                    op0=Alu.logical_shift_right,
                )
                lo_i = idxp.tile([128, NCHUNK], i32, tag="lo")
                nc.vector.tensor_scalar(
                    out=lo_i[:], in0=ids_t[:], scalar1=MASK, scalar2=None,
                    op0=Alu.bitwise_and,
                )
                hi_h = idxp.tile([128, NCHUNK], f32, tag="hih")
                nc.vector.tensor_copy(out=hi_h[:], in_=hi_i[:])
                lo_h = idxp.tile([128, NCHUNK], f32, tag="loh")
                nc.vector.tensor_copy(out=lo_h[:], in_=lo_i[:])

                gt = gtp.tile([128, NCHUNK * PAIRS], f16, tag="gt")
                alo = gtp.tile([128, NCHUNK * V2], f16, tag="alo")
                for c in range(NCHUNK):
                    pt = psT.tile([128, O], f32, tag="pt")
                    nc.tensor.transpose(
                        out=pt[:],
                        in_=attn_s[:, c * 128 : (c + 1) * 128],
                        identity=iden[:O, :O],
                    )
                    at = idxp.tile([128, O], f16, tag="at")
                    nc.vector.tensor_copy(out=at[:], in_=pt[:])

                    bh = idxp.tile([128, PAIRS], f16, tag="bh")
                    nc.vector.tensor_scalar(
                        out=bh[:], in0=gidx[:], scalar1=hi_h[:, c : c + 1],
                        scalar2=None, op0=Alu.is_equal,
                    )
                    nc.vector.tensor_tensor(
                        out=gt[:, c * PAIRS : (c + 1) * PAIRS].rearrange(
                            "p (g o) -> p g o", o=O
                        ),
                        in0=at[:].unsqueeze(1).broadcast_to([128, V1, O]),
                        in1=bh[:].rearrange("p (g o) -> p g o", o=O),
                        op=Alu.mult,
                    )
                    nc.vector.tensor_scalar(
                        out=alo[:, c * V2 : (c + 1) * V2], in0=lov[:],
                        scalar1=lo_h[:, c : c + 1], scalar2=None, op0=Alu.is_equal,
                    )

                for t in range(NT_FULL + 1):
                    w = 128 if t < NT_FULL else LAST_W
                    ps = psmm.tile([128, V2], f32, tag="mm")
                    for h in range(V2 // 512):
                        for c in range(NCHUNK):
                            nc.tensor.matmul(
                                out=ps[:w, h * 512 : (h + 1) * 512],
                                lhsT=gt[:, c * PAIRS + t * 128 : c * PAIRS + t * 128 + w],
                                rhs=alo[:, c * V2 + h * 512 : c * V2 + h * 512 + 512],
                                start=(c == 0),
                                stop=(c == NCHUNK - 1),
                            )
                    os_ = outp.tile([128, V2], f32, tag="os")
                    eng = nc.scalar if t % 2 == 0 else nc.vector
                    if eng is nc.scalar:
                        eng.copy(out=os_[:w], in_=ps[:w])
                    else:
                        eng.tensor_copy(out=os_[:w], in_=ps[:w])
                    if t < NT_FULL:
                        nc.sync.dma_start(
                            out=out_d[b][:, t * 4 * V2 : (t + 1) * 4 * V2].rearrange(
                                "o (g l) -> g o l", l=V2
                            ),
                            in_=os_[:],
                        )
                    else:
                        nc.sync.dma_start(
                            out=out_d[b][:, (V1 - 1) * V2 : SIZE],
                            in_=os_[:LAST_W, :LAST_VALID],
                        )

    nc.compile()
    return nc


def _consts():
    gidx = np.broadcast_to(
        np.repeat(np.arange(V1, dtype=np.float16), O)[None, :], (128, PAIRS)
    ).copy()
    lov = np.broadcast_to(
        np.arange(V2, dtype=np.float16)[None, :], (128, V2)
    ).copy()
    iden = np.eye(128, dtype=np.float32)
    return gidx, lov, iden


def kernel(ids, attn):
    from concourse.bass_utils import run_bass_kernel_spmd

    ids = np.ascontiguousarray(ids, dtype=np.int32)
    attn = np.ascontiguousarray(attn, dtype=np.float32)

    if "nc" not in _cache:
        _cache["nc"] = _build()
    nc = _cache["nc"]

    gidx, lov, iden = _consts()
    core_ids = list(range(NCORES))
    in_maps = [
        {
            "ids": ids[c * BPC : (c + 1) * BPC],
            "attn": attn[c * BPC : (c + 1) * BPC],
            "gidx": gidx,
            "lov": lov,
            "iden": iden,
        }
        for c in core_ids
    ]
    res = run_bass_kernel_spmd(nc, in_maps, core_ids)
    out = np.concatenate([res.results[c]["out"] for c in core_ids], axis=0)
    return out


# revision 4
# speedup vs baseline: 1.9149x; 1.9149x over previous
"""Trainium2 Bass kernel for batched weighted scatter-add (AttentionCopy).

Computes out[b, o, v] = sum_i attn[b, o, i] * (ids[b, i] == v)
for ids [16, 512] int32 in [0, 50000), attn [16, 32, 512] f32,
out [16, 32, 50000] f32.

Strategy: pure data parallel over the batch dim — 2 batches per core on 8
cores. Per batch the output is built *densely* on-device via a two-level
one-hot factorization of the vocab index (v = g*2000 + lo, 25 groups):

  out[o, g, lo] = sum_i (attnT[i, o] * (hi_i == g)) * (lo_i == lo)
                = (G_(o,g)).T @ Alo      (PE matmul, contraction over i)

The masked matrix G.T[i, (o, g)] and the one-hot Alo[i, lo] are built with
iota-constant compares on the vector engine. With o-major (o, g) pairs and
V2 = 2000 (25 * 2000 = 50000 exactly), each [128, 2000] f32 PSUM tile maps
to a *flat contiguous* 1 MB run of the row-major [32, 50000] output, so
every DRAM write is a full-partition coalesced DMA (all 16 SDMA engines).

hi = ids // 2000 uses the exact integer magic (ids >> 4) * 4195 >> 19
(validated exhaustively for [0, 50000), incl. fp32-rounded intermediates).
"""

import sys

sys.path.insert(0, "/opt/trn_rl_repo")

import numpy as np

NCORES = 8
B, O, I = 16, 32, 512
SIZE = 50000
BPC = B // NCORES  # batches per core
V2 = 2000  # lo range (4 PSUM banks per tile)
V1 = 25  # number of vocab groups: 25 * 2000 = 50000 exactly
PAIRS = O * V1  # 800 (o, g) pairs per batch, o-major
NT_FULL = PAIRS // 128  # 6 full 128-pair tiles
LAST_W = PAIRS - NT_FULL * 128  # 32 pairs in the final tile
NCHUNK = I // 128  # 4 contraction chunks
# matmul N-slices of V2, each within one 2 KiB PSUM bank
NSLICES = [(0, 512), (512, 1024), (1024, 1536), (1536, 2000)]

_cache = {}


def _build(mm_dtype="bfloat16"):
    import concourse.bacc as bacc
    import concourse.mybir as mybir
    import concourse.tile as tile

    f32 = mybir.dt.float32
    f16 = mybir.dt.float16
    mmdt = getattr(mybir.dt, mm_dtype)
    i32 = mybir.dt.int32
    Alu = mybir.AluOpType

    nc = bacc.Bacc("TRN2", target_bir_lowering=False, debug=False, num_devices=NCORES)

    # ids pre-tiled on host to [BPC, 128, NCHUNK] with [p, c] = ids[c*128 + p]
    ids_d = nc.dram_tensor("ids", [BPC, 128, NCHUNK], i32, kind="ExternalInput").ap()
    attn_d = nc.dram_tensor("attn", [BPC, O, I], f32, kind="ExternalInput").ap()
    gidx_d = nc.dram_tensor("gidx", [128, PAIRS], f16, kind="ExternalInput").ap()
    lov_d = nc.dram_tensor("lov", [128, V2], f16, kind="ExternalInput").ap()
    out_d = nc.dram_tensor("out", [BPC, O, SIZE], f32, kind="ExternalOutput").ap()

    with tile.TileContext(nc) as tc:
        with (
            tc.tile_pool(name="const", bufs=1) as constp,
            tc.tile_pool(name="inp", bufs=2) as inp,
            tc.tile_pool(name="idx", bufs=2) as idxp,
            tc.tile_pool(name="gt", bufs=2) as gtp,
            tc.tile_pool(name="outs", bufs=4) as outp,
            tc.tile_pool(name="psmm", bufs=2, space="PSUM") as psmm,
        ):
            gidx = constp.tile([128, PAIRS], f16)
            nc.sync.dma_start(out=gidx[:], in_=gidx_d[:])
            lov = constp.tile([128, V2], f16)
            nc.sync.dma_start(out=lov[:], in_=lov_d[:])

            for b in range(BPC):
                ids_t = idxp.tile([128, NCHUNK], i32, tag="ids")
                nc.sync.dma_start(out=ids_t[:], in_=ids_d[b])
                attn_s = inp.tile([O, I], f32, tag="attn")
                nc.sync.dma_start(out=attn_s[:], in_=attn_d[b])

                # hi = ids // 2000 via exact magic; lo = ids - 2000*hi
                t1 = idxp.tile([128, NCHUNK], i32, tag="t1")
                nc.vector.tensor_scalar(out=t1[:], in0=ids_t[:], scalar1=4,
                                        scalar2=None, op0=Alu.logical_shift_right)
                t2 = idxp.tile([128, NCHUNK], i32, tag="t2")
                nc.vector.tensor_scalar(out=t2[:], in0=t1[:], scalar1=4195,
                                        scalar2=None, op0=Alu.mult)
                hi_i = idxp.tile([128, NCHUNK], i32, tag="hi")
                nc.vector.tensor_scalar(out=hi_i[:], in0=t2[:], scalar1=19,
                                        scalar2=None, op0=Alu.logical_shift_right)
                hi2k = idxp.tile([128, NCHUNK], i32, tag="hi2k")
                nc.vector.tensor_scalar(out=hi2k[:], in0=hi_i[:], scalar1=2000,
                                        scalar2=None, op0=Alu.mult)
                lo_i = idxp.tile([128, NCHUNK], i32, tag="lo")
                nc.vector.tensor_sub(out=lo_i[:], in0=ids_t[:], in1=hi2k[:])
                hi_f = idxp.tile([128, NCHUNK], f32, tag="hif")
                nc.vector.tensor_copy(out=hi_f[:], in_=hi_i[:])
                lo_f = idxp.tile([128, NCHUNK], f32, tag="lof")
                nc.vector.tensor_copy(out=lo_f[:], in_=lo_i[:])

                gt = gtp.tile([128, NCHUNK * PAIRS], mmdt, tag="gt")
                alo = gtp.tile([128, NCHUNK * V2], mmdt, tag="alo")
                for c in range(NCHUNK):
                    # attnT chunk via DVE 32x32 block transposes (SBUF->SBUF)
                    at = idxp.tile([128, O], f32, tag="at")
                    for j in range(4):
                        nc.vector.transpose(
                            out=at[32 * j : 32 * j + 32, :],
                            in_=attn_s[:, c * 128 + 32 * j : c * 128 + 32 * j + 32],
                        )
                    bh = idxp.tile([128, PAIRS], mmdt, tag="bh")
                    nc.vector.tensor_scalar(out=bh[:], in0=gidx[:],
                                            scalar1=hi_f[:, c : c + 1],
                                            scalar2=None, op0=Alu.is_equal)
                    nc.vector.tensor_tensor(
                        out=gt[:, c * PAIRS : (c + 1) * PAIRS].rearrange(
                            "p (o g) -> p o g", g=V1
                        ),
                        in0=at[:].unsqueeze(2).broadcast_to([128, O, V1]),
                        in1=bh[:].rearrange("p (o g) -> p o g", g=V1),
                        op=Alu.mult,
                    )
                    nc.vector.tensor_scalar(out=alo[:, c * V2 : (c + 1) * V2],
                                            in0=lov[:],
                                            scalar1=lo_f[:, c : c + 1],
                                            scalar2=None, op0=Alu.is_equal)

                out_flat = out_d[b].rearrange("o v -> (o v)")
                for t in range(NT_FULL + 1):
                    w = 128 if t < NT_FULL else LAST_W
                    ps = psmm.tile([128, V2], f32, tag="mm")
                    for n0, n1 in NSLICES:
                        for c in range(NCHUNK):
                            nc.tensor.matmul(
                                out=ps[:w, n0:n1],
                                lhsT=gt[:, c * PAIRS + t * 128 : c * PAIRS + t * 128 + w],
                                rhs=alo[:, c * V2 + n0 : c * V2 + n1],
                                start=(c == 0),
                                stop=(c == NCHUNK - 1),
                            )
                    os_ = outp.tile([128, V2], f32, tag="os")
                    eng = nc.scalar if t % 2 == 0 else nc.vector
                    if eng is nc.scalar:
                        eng.copy(out=os_[:w], in_=ps[:w])
                    else:
                        eng.tensor_copy(out=os_[:w], in_=ps[:w])
                    nc.sync.dma_start(
                        out=out_flat[t * 128 * V2 : t * 128 * V2 + w * V2].rearrange(
                            "(p l) -> p l", l=V2
                        ),
                        in_=os_[:w, :],
                    )

    nc.compile()
    return nc


def _consts():
    gidx = np.broadcast_to(
        np.tile(np.arange(V1, dtype=np.float16), O)[None, :], (128, PAIRS)
    ).copy()
    lov = np.broadcast_to(
        np.arange(V2, dtype=np.float16)[None, :], (128, V2)
    ).copy()
    return gidx, lov


def kernel(ids, attn):
    from concourse.bass_utils import run_bass_kernel_spmd

    ids = np.ascontiguousarray(ids, dtype=np.int32)
    attn = np.ascontiguousarray(attn, dtype=np.float32)

    if "nc" not in _cache:
        _cache["nc"] = _build()
    nc = _cache["nc"]

    gidx, lov = _consts()
    # [B, I] -> per core [BPC, 128, NCHUNK] with [b, p, c] = ids[b, c*128 + p]
    ids_t = ids.reshape(B, NCHUNK, 128).transpose(0, 2, 1)
    core_ids = list(range(NCORES))
    in_maps = [
        {
            "ids": np.ascontiguousarray(ids_t[c * BPC : (c + 1) * BPC]),
            "attn": attn[c * BPC : (c + 1) * BPC],
            "gidx": gidx,
            "lov": lov,
        }
        for c in core_ids
    ]
    res = run_bass_kernel_spmd(nc, in_maps, core_ids)
    out = np.concatenate([res.results[c]["out"] for c in core_ids], axis=0)
    return out


# revision 6
# speedup vs baseline: 2.0560x; 1.0737x over previous
"""Trainium2 Bass kernel for batched weighted scatter-add (AttentionCopy).

Computes out[b, o, v] = sum_i attn[b, o, i] * (ids[b, i] == v)
for ids [16, 512] int32 in [0, 50000), attn [16, 32, 512] f32,
out [16, 32, 50000] f32.

Strategy: pure data parallel over the batch dim — 2 batches per core on 8
cores. Per batch the output is built *densely* on-device via a two-level
one-hot factorization of the vocab index (v = g*2000 + lo, 25 groups):

  out[o, g, lo] = sum_i (attnT[i, o] * (hi_i == g)) * (lo_i == lo)
                = (G_(o,g)).T @ Alo      (PE matmul, contraction over i)

The masked matrix G.T[i, (o, g)] and the one-hot Alo[i, lo] are built with
iota-constant compares on the vector engine. With o-major (o, g) pairs and
V2 = 2000 (25 * 2000 = 50000 exactly), each [128, 2000] f32 PSUM tile maps
to a *flat contiguous* 1 MB run of the row-major [32, 50000] output, so
every DRAM write is a full-partition coalesced DMA (all 16 SDMA engines).

hi = ids // 2000 uses the exact integer magic (ids >> 4) * 4195 >> 19
(validated exhaustively for [0, 50000), incl. fp32-rounded intermediates).
"""

import sys

sys.path.insert(0, "/opt/trn_rl_repo")

import numpy as np

NCORES = 8
B, O, I = 16, 32, 512
SIZE = 50000
BPC = B // NCORES  # batches per core
V2 = 2000  # lo range (4 PSUM banks per tile)
V1 = 25  # number of vocab groups: 25 * 2000 = 50000 exactly
PAIRS = O * V1  # 800 (o, g) pairs per batch, o-major
NT_FULL = PAIRS // 128  # 6 full 128-pair tiles
LAST_W = PAIRS - NT_FULL * 128  # 32 pairs in the final tile
NCHUNK = I // 128  # 4 contraction chunks
# matmul N-slices of V2, each within one 2 KiB PSUM bank
NSLICES = [(0, 512), (512, 1024), (1024, 1536), (1536, 2000)]

_cache = {}


def _build(mm_dtype="bfloat16"):
    import concourse.bacc as bacc
    import concourse.mybir as mybir
    import concourse.tile as tile

    f32 = mybir.dt.float32
    f16 = mybir.dt.float16
    mmdt = getattr(mybir.dt, mm_dtype)
    i32 = mybir.dt.int32
    Alu = mybir.AluOpType

    nc = bacc.Bacc("TRN2", target_bir_lowering=False, debug=False, num_devices=NCORES)

    # ids pre-tiled on host to [BPC, 128, NCHUNK] with [p, c] = ids[c*128 + p]
    ids_d = nc.dram_tensor("ids", [BPC, 128, NCHUNK], i32, kind="ExternalInput").ap()
    attn_d = nc.dram_tensor("attn", [BPC, I, O], f32, kind="ExternalInput").ap()
    gidx_d = nc.dram_tensor("gidx", [128, PAIRS], f16, kind="ExternalInput").ap()
    lov_d = nc.dram_tensor("lov", [128, V2], f16, kind="ExternalInput").ap()
    out_d = nc.dram_tensor("out", [BPC, O, SIZE], f32, kind="ExternalOutput").ap()

    with tile.TileContext(nc) as tc:
        with (
            tc.tile_pool(name="const", bufs=1) as constp,
            tc.tile_pool(name="inp", bufs=2) as inp,
            tc.tile_pool(name="idx", bufs=2) as idxp,
            tc.tile_pool(name="gt", bufs=2) as gtp,
            tc.tile_pool(name="outs", bufs=4) as outp,
            tc.tile_pool(name="psmm", bufs=2, space="PSUM") as psmm,
        ):
            warm = constp.tile([128, 256], mmdt)
            nc.gpsimd.memset(warm[:], 0)
            wps = psmm.tile([128, 256], f32, tag="mm")
            for _ in range(64):
                nc.tensor.matmul(out=wps[:, :256], lhsT=warm[:, :128],
                                 rhs=warm[:, :256], start=True, stop=True)
            gidx = constp.tile([128, PAIRS], f16)
            nc.sync.dma_start(out=gidx[:], in_=gidx_d[:])
            lov = constp.tile([128, V2], f16)
            nc.sync.dma_start(out=lov[:], in_=lov_d[:])

            for b in range(BPC):
                ids_t = idxp.tile([128, NCHUNK], i32, tag="ids")
                nc.sync.dma_start(out=ids_t[:], in_=ids_d[b])
                at_all = inp.tile([128, NCHUNK * O], f32, tag="attn")
                nc.sync.dma_start(
                    out=at_all[:].rearrange("p (c o) -> p c o", o=O),
                    in_=attn_d[b].rearrange("(c p) o -> p c o", p=128),
                )

                # hi = ids // 2000 via exact magic; lo = ids - 2000*hi
                t1 = idxp.tile([128, NCHUNK], i32, tag="t1")
                nc.vector.tensor_scalar(out=t1[:], in0=ids_t[:], scalar1=4,
                                        scalar2=None, op0=Alu.logical_shift_right)
                t2 = idxp.tile([128, NCHUNK], i32, tag="t2")
                nc.vector.tensor_scalar(out=t2[:], in0=t1[:], scalar1=4195,
                                        scalar2=None, op0=Alu.mult)
                hi_i = idxp.tile([128, NCHUNK], i32, tag="hi")
                nc.vector.tensor_scalar(out=hi_i[:], in0=t2[:], scalar1=19,
                                        scalar2=None, op0=Alu.logical_shift_right)
                hi2k = idxp.tile([128, NCHUNK], i32, tag="hi2k")
                nc.vector.tensor_scalar(out=hi2k[:], in0=hi_i[:], scalar1=2000,
                                        scalar2=None, op0=Alu.mult)
                lo_i = idxp.tile([128, NCHUNK], i32, tag="lo")
                nc.vector.tensor_sub(out=lo_i[:], in0=ids_t[:], in1=hi2k[:])
                hi_f = idxp.tile([128, NCHUNK], f32, tag="hif")
                nc.vector.tensor_copy(out=hi_f[:], in_=hi_i[:])
                lo_f = idxp.tile([128, NCHUNK], f32, tag="lof")
                nc.vector.tensor_copy(out=lo_f[:], in_=lo_i[:])

                gt = gtp.tile([128, NCHUNK * PAIRS], mmdt, tag="gt")
                alo = gtp.tile([128, NCHUNK * V2], mmdt, tag="alo")
                for c in range(NCHUNK):
                    at = at_all[:, c * O : (c + 1) * O]
                    bh = idxp.tile([128, PAIRS], mmdt, tag="bh")
                    nc.vector.tensor_scalar(out=bh[:], in0=gidx[:],
                                            scalar1=hi_f[:, c : c + 1],
                                            scalar2=None, op0=Alu.is_equal)
                    nc.vector.tensor_tensor(
                        out=gt[:, c * PAIRS : (c + 1) * PAIRS].rearrange(
                            "p (o g) -> p o g", g=V1
                        ),
                        in0=at.unsqueeze(2).broadcast_to([128, O, V1]),
                        in1=bh[:].rearrange("p (o g) -> p o g", g=V1),
                        op=Alu.mult,
                    )
                    nc.vector.tensor_scalar(out=alo[:, c * V2 : (c + 1) * V2],
                                            in0=lov[:],
                                            scalar1=lo_f[:, c : c + 1],
                                            scalar2=None, op0=Alu.is_equal)

                out_flat = out_d[b].rearrange("o v -> (o v)")
                for t in range(NT_FULL + 1):
                    w = 128 if t < NT_FULL else LAST_W
                    ps = psmm.tile([128, V2], f32, tag="mm")
                    for n0, n1 in NSLICES:
                        for c in range(NCHUNK):
                            nc.tensor.matmul(
                                out=ps[:w, n0:n1],
                                lhsT=gt[:, c * PAIRS + t * 128 : c * PAIRS + t * 128 + w],
                                rhs=alo[:, c * V2 + n0 : c * V2 + n1],
                                start=(c == 0),
                                stop=(c == NCHUNK - 1),
                            )
                    os_ = outp.tile([128, V2], f32, tag="os")
                    eng = nc.scalar if t % 2 == 0 else nc.vector
                    if eng is nc.scalar:
                        eng.copy(out=os_[:w], in_=ps[:w])
                    else:
                        eng.tensor_copy(out=os_[:w], in_=ps[:w])
                    nc.sync.dma_start(
                        out=out_flat[t * 128 * V2 : t * 128 * V2 + w * V2].rearrange(
                            "(p l) -> p l", l=V2
                        ),
                        in_=os_[:w, :],
                    )

    nc.compile()
    return nc


def _consts():
    gidx = np.broadcast_to(
        np.tile(np.arange(V1, dtype=np.float16), O)[None, :], (128, PAIRS)
    ).copy()
    lov = np.broadcast_to(
        np.arange(V2, dtype=np.float16)[None, :], (128, V2)
    ).copy()
    return gidx, lov


def kernel(ids, attn):
    from concourse.bass_utils import run_bass_kernel_spmd

    ids = np.ascontiguousarray(ids, dtype=np.int32)
    attn = np.ascontiguousarray(attn, dtype=np.float32)

    if "nc" not in _cache:
        _cache["nc"] = _build()
    nc = _cache["nc"]

    gidx, lov = _consts()
    # [B, I] -> per core [BPC, 128, NCHUNK] with [b, p, c] = ids[b, c*128 + p]
    ids_t = ids.reshape(B, NCHUNK, 128).transpose(0, 2, 1)
    attn_t = attn.transpose(0, 2, 1)  # [B, I, O]
    core_ids = list(range(NCORES))
    in_maps = [
        {
            "ids": np.ascontiguousarray(ids_t[c * BPC : (c + 1) * BPC]),
            "attn": np.ascontiguousarray(attn_t[c * BPC : (c + 1) * BPC]),
            "gidx": gidx,
            "lov": lov,
        }
        for c in core_ids
    ]
    res = run_bass_kernel_spmd(nc, in_maps, core_ids)
    out = np.concatenate([res.results[c]["out"] for c in core_ids], axis=0)
    return out


# revision 7
# speedup vs baseline: 2.1517x; 1.0465x over previous
"""Trainium2 Bass kernel for batched weighted scatter-add (AttentionCopy).

Computes out[b, o, v] = sum_i attn[b, o, i] * (ids[b, i] == v)
for ids [16, 512] int32 in [0, 50000), attn [16, 32, 512] f32,
out [16, 32, 50000] f32.

Strategy: pure data parallel over the batch dim — 2 batches per core on 8
cores. Per batch the output is built *densely* on-device via a two-level
one-hot factorization of the vocab index (v = g*2000 + lo, 25 groups):

  out[o, g, lo] = sum_i (attnT[i, o] * (hi_i == g)) * (lo_i == lo)
                = (G_(o,g)).T @ Alo      (PE matmul, contraction over i)

The masked matrix G.T[i, (o, g)] and the one-hot Alo[i, lo] are built with
iota-constant compares on the vector engine. With o-major (o, g) pairs and
V2 = 2000 (25 * 2000 = 50000 exactly), each [128, 2000] f32 PSUM tile maps
to a *flat contiguous* 1 MB run of the row-major [32, 50000] output, so
every DRAM write is a full-partition coalesced DMA (all 16 SDMA engines).

hi = ids // 2000 uses the exact integer magic (ids >> 4) * 4195 >> 19
(validated exhaustively for [0, 50000), incl. fp32-rounded intermediates).
"""

import sys

sys.path.insert(0, "/opt/trn_rl_repo")

import numpy as np

NCORES = 8
B, O, I = 16, 32, 512
SIZE = 50000
BPC = B // NCORES  # batches per core
V2 = 2000  # lo range (4 PSUM banks per tile)
V1 = 25  # number of vocab groups: 25 * 2000 = 50000 exactly
PAIRS = O * V1  # 800 (o, g) pairs per batch, o-major
NT_FULL = PAIRS // 128  # 6 full 128-pair tiles
LAST_W = PAIRS - NT_FULL * 128  # 32 pairs in the final tile
NCHUNK = I // 128  # 4 contraction chunks
# matmul N-slices of V2, each within one 2 KiB PSUM bank
NSLICES = [(0, 512), (512, 1024), (1024, 1536), (1536, 2000)]

_cache = {}


def _build(mm_dtype="bfloat16"):
    import concourse.bacc as bacc
    import concourse.mybir as mybir
    import concourse.tile as tile

    f32 = mybir.dt.float32
    f16 = mybir.dt.float16
    mmdt = getattr(mybir.dt, mm_dtype)
    i32 = mybir.dt.int32
    Alu = mybir.AluOpType

    nc = bacc.Bacc("TRN2", target_bir_lowering=False, debug=False, num_devices=NCORES)

    # ids pre-tiled on host to [BPC, 128, NCHUNK] with [p, c] = ids[c*128 + p]
    ids_d = nc.dram_tensor("ids", [BPC, 128, NCHUNK], i32, kind="ExternalInput").ap()
    attn_d = nc.dram_tensor("attn", [BPC, I, O], f32, kind="ExternalInput").ap()
    gidx_d = nc.dram_tensor("gidx", [128, PAIRS], f16, kind="ExternalInput").ap()
    lov_d = nc.dram_tensor("lov", [128, V2], f16, kind="ExternalInput").ap()
    out_d = nc.dram_tensor("out", [BPC, O, SIZE], f32, kind="ExternalOutput").ap()

    with tile.TileContext(nc) as tc:
        with (
            tc.tile_pool(name="const", bufs=1) as constp,
            tc.tile_pool(name="inp", bufs=2) as inp,
            tc.tile_pool(name="idx", bufs=2) as idxp,
            tc.tile_pool(name="gt", bufs=2) as gtp,
            tc.tile_pool(name="outs", bufs=4) as outp,
            tc.tile_pool(name="psmm", bufs=2, space="PSUM") as psmm,
        ):
            warm = constp.tile([128, 256], mmdt)
            nc.gpsimd.memset(warm[:], 0)
            wps = psmm.tile([128, 256], f32, tag="mm")
            for _ in range(64):
                nc.tensor.matmul(out=wps[:, :256], lhsT=warm[:, :128],
                                 rhs=warm[:, :256], start=True, stop=True)
            gidx = constp.tile([128, PAIRS], f16)
            nc.sync.dma_start(out=gidx[:], in_=gidx_d[:])
            lov = constp.tile([128, V2], f16)
            nc.sync.dma_start(out=lov[:], in_=lov_d[:])

            for b in range(BPC):
                ids_t = idxp.tile([128, NCHUNK], i32, tag="ids")
                nc.sync.dma_start(out=ids_t[:], in_=ids_d[b])
                at_all = inp.tile([128, NCHUNK * O], f32, tag="attn")
                nc.sync.dma_start(
                    out=at_all[:].rearrange("p (c o) -> p c o", o=O),
                    in_=attn_d[b].rearrange("(c p) o -> p c o", p=128),
                )

                # hi = ids // 2000 via exact magic; lo = ids - 2000*hi
                t1 = idxp.tile([128, NCHUNK], i32, tag="t1")
                nc.vector.tensor_scalar(out=t1[:], in0=ids_t[:], scalar1=4,
                                        scalar2=None, op0=Alu.logical_shift_right)
                t2 = idxp.tile([128, NCHUNK], i32, tag="t2")
                nc.vector.tensor_scalar(out=t2[:], in0=t1[:], scalar1=4195,
                                        scalar2=None, op0=Alu.mult)
                hi_i = idxp.tile([128, NCHUNK], i32, tag="hi")
                nc.vector.tensor_scalar(out=hi_i[:], in0=t2[:], scalar1=19,
                                        scalar2=None, op0=Alu.logical_shift_right)
                hi2k = idxp.tile([128, NCHUNK], i32, tag="hi2k")
                nc.vector.tensor_scalar(out=hi2k[:], in0=hi_i[:], scalar1=2000,
                                        scalar2=None, op0=Alu.mult)
                lo_i = idxp.tile([128, NCHUNK], i32, tag="lo")
                nc.vector.tensor_sub(out=lo_i[:], in0=ids_t[:], in1=hi2k[:])
                hi_f = idxp.tile([128, NCHUNK], f32, tag="hif")
                nc.vector.tensor_copy(out=hi_f[:], in_=hi_i[:])
                lo_f = idxp.tile([128, NCHUNK], f32, tag="lof")
                nc.vector.tensor_copy(out=lo_f[:], in_=lo_i[:])

                gt = gtp.tile([128, NCHUNK * PAIRS], mmdt, tag="gt")
                alo = gtp.tile([128, NCHUNK * V2], mmdt, tag="alo")
                for c in range(NCHUNK):
                    at = at_all[:, c * O : (c + 1) * O]
                    nc.vector.tensor_scalar(out=alo[:, c * V2 : (c + 1) * V2],
                                            in0=lov[:],
                                            scalar1=lo_f[:, c : c + 1],
                                            scalar2=None, op0=Alu.is_equal)
                    bh = idxp.tile([128, PAIRS], mmdt, tag="bh")
                    nc.vector.tensor_scalar(out=bh[:], in0=gidx[:],
                                            scalar1=hi_f[:, c : c + 1],
                                            scalar2=None, op0=Alu.is_equal)
                    # split gt by pair-halves so early tiles' matmuls unblock
                    # before the full chunk is built
                    for p0, p1 in ((0, 16), (16, O)):
                        nc.vector.tensor_tensor(
                            out=gt[:, c * PAIRS + p0 * V1 : c * PAIRS + p1 * V1]
                            .rearrange("p (o g) -> p o g", g=V1),
                            in0=at[:, p0:p1].unsqueeze(2).broadcast_to(
                                [128, p1 - p0, V1]),
                            in1=bh[:, p0 * V1 : p1 * V1].rearrange(
                                "p (o g) -> p o g", g=V1),
                            op=Alu.mult,
                        )

                out_flat = out_d[b].rearrange("o v -> (o v)")
                for t in range(NT_FULL + 1):
                    w = 128 if t < NT_FULL else LAST_W
                    ps = psmm.tile([128, V2], f32, tag="mm")
                    for c in range(NCHUNK):
                        for n0, n1 in NSLICES:
                            nc.tensor.matmul(
                                out=ps[:w, n0:n1],
                                lhsT=gt[:, c * PAIRS + t * 128 : c * PAIRS + t * 128 + w],
                                rhs=alo[:, c * V2 + n0 : c * V2 + n1],
                                start=(c == 0),
                                stop=(c == NCHUNK - 1),
                            )
                    os_ = outp.tile([128, V2], f32, tag="os")
                    last = b == BPC - 1 and t == NT_FULL
                    halves = ((0, V2 // 2), (V2 // 2, V2)) if last else ((0, V2),)
                    for k, (v0, v1) in enumerate(halves):
                        if (t + k) % 2 == 0:
                            nc.scalar.copy(out=os_[:w, v0:v1], in_=ps[:w, v0:v1])
                        else:
                            nc.vector.tensor_copy(out=os_[:w, v0:v1],
                                                  in_=ps[:w, v0:v1])
                        nc.sync.dma_start(
                            out=out_flat[t * 128 * V2 : t * 128 * V2 + w * V2]
                            .rearrange("(p l) -> p l", l=V2)[:, v0:v1],
                            in_=os_[:w, v0:v1],
                        )

    nc.compile()
    return nc


def _consts():
    gidx = np.broadcast_to(
        np.tile(np.arange(V1, dtype=np.float16), O)[None, :], (128, PAIRS)
    ).copy()
    lov = np.broadcast_to(
        np.arange(V2, dtype=np.float16)[None, :], (128, V2)
    ).copy()
    return gidx, lov


def kernel(ids, attn):
    from concourse.bass_utils import run_bass_kernel_spmd

    ids = np.ascontiguousarray(ids, dtype=np.int32)
    attn = np.ascontiguousarray(attn, dtype=np.float32)

    if "nc" not in _cache:
        _cache["nc"] = _build()
    nc = _cache["nc"]

    gidx, lov = _consts()
    # [B, I] -> per core [BPC, 128, NCHUNK] with [b, p, c] = ids[b, c*128 + p]
    ids_t = ids.reshape(B, NCHUNK, 128).transpose(0, 2, 1)
    attn_t = attn.transpose(0, 2, 1)  # [B, I, O]
    core_ids = list(range(NCORES))
    in_maps = [
        {
            "ids": np.ascontiguousarray(ids_t[c * BPC : (c + 1) * BPC]),
            "attn": np.ascontiguousarray(attn_t[c * BPC : (c + 1) * BPC]),
            "gidx": gidx,
            "lov": lov,
        }
        for c in core_ids
    ]
    res = run_bass_kernel_spmd(nc, in_maps, core_ids)
    out = np.concatenate([res.results[c]["out"] for c in core_ids], axis=0)
    return out


# revision 8
# speedup vs baseline: 2.1800x; 1.0132x over previous
"""Trainium2 Bass kernel for batched weighted scatter-add (AttentionCopy).

Computes out[b, o, v] = sum_i attn[b, o, i] * (ids[b, i] == v)
for ids [16, 512] int32 in [0, 50000), attn [16, 32, 512] f32,
out [16, 32, 50000] f32.

Strategy: pure data parallel over the batch dim — 2 batches per core on 8
cores. Per batch the output is built *densely* on-device via a two-level
one-hot factorization of the vocab index (v = g*2000 + lo, 25 groups):

  out[o, g, lo] = sum_i (attnT[i, o] * (hi_i == g)) * (lo_i == lo)
                = (G_(o,g)).T @ Alo      (PE matmul, contraction over i)

The masked matrix G.T[i, (o, g)] and the one-hot Alo[i, lo] are built with
iota-constant compares on the vector engine. With o-major (o, g) pairs and
V2 = 1250 (40 * 1250 = 50000 exactly), each [128, 1250] f32 PSUM tile maps
to a *flat contiguous* 640 KB run of the row-major [32, 50000] output, so
every DRAM write is a full-partition coalesced DMA (all 16 SDMA engines),
and the 1280 pairs split into exactly 10 full 128-partition tiles.

hi = ids // 1250 uses the round-to-nearest int cast of
(ids + 0.5) * (1/1250) - 0.5, validated exhaustively on HW for [0, 50000).
"""

import sys

sys.path.insert(0, "/opt/trn_rl_repo")

import numpy as np

NCORES = 8
B, O, I = 16, 32, 512
SIZE = 50000
BPC = B // NCORES  # batches per core
V2 = 1250  # lo range (3 PSUM banks per tile)
V1 = 40  # number of vocab groups: 40 * 1250 = 50000 exactly
PAIRS = O * V1  # 1280 (o, g) pairs per batch, o-major
NTILES = PAIRS // 128  # exactly 10 full 128-pair tiles
NCHUNK = I // 128  # 4 contraction chunks
# matmul N-slices of V2, each within one 2 KiB PSUM bank
NSLICES = [(0, 512), (512, 1024), (1024, 1250)]

_cache = {}


def _build(mm_dtype="bfloat16"):
    import concourse.bacc as bacc
    import concourse.mybir as mybir
    import concourse.tile as tile

    f32 = mybir.dt.float32
    f16 = mybir.dt.float16
    mmdt = getattr(mybir.dt, mm_dtype)
    i32 = mybir.dt.int32
    Alu = mybir.AluOpType

    nc = bacc.Bacc("TRN2", target_bir_lowering=False, debug=False, num_devices=NCORES)

    # ids pre-tiled on host to [BPC, 128, NCHUNK] with [p, c] = ids[c*128 + p]
    ids_d = nc.dram_tensor("ids", [BPC, 128, NCHUNK], i32, kind="ExternalInput").ap()
    attn_d = nc.dram_tensor("attn", [BPC, I, O], f32, kind="ExternalInput").ap()
    gidx_d = nc.dram_tensor("gidx", [128, PAIRS], f16, kind="ExternalInput").ap()
    lov_d = nc.dram_tensor("lov", [128, V2], f16, kind="ExternalInput").ap()
    out_d = nc.dram_tensor("out", [BPC, O, SIZE], f32, kind="ExternalOutput").ap()

    with tile.TileContext(nc) as tc:
        with (
            tc.tile_pool(name="const", bufs=1) as constp,
            tc.tile_pool(name="inp", bufs=2) as inp,
            tc.tile_pool(name="idx", bufs=2) as idxp,
            tc.tile_pool(name="gt", bufs=2) as gtp,
            tc.tile_pool(name="outs", bufs=4) as outp,
            tc.tile_pool(name="psmm", bufs=2, space="PSUM") as psmm,
        ):
            warm = constp.tile([128, 256], mmdt)
            nc.gpsimd.memset(warm[:], 0)
            wps = psmm.tile([128, 256], f32, tag="mm")
            for _ in range(36):
                nc.tensor.matmul(out=wps[:, :256], lhsT=warm[:, :128],
                                 rhs=warm[:, :256], start=True, stop=True)
            gidx = constp.tile([128, PAIRS], f16)
            nc.sync.dma_start(out=gidx[:], in_=gidx_d[:])
            lov = constp.tile([128, V2], f16)
            nc.sync.dma_start(out=lov[:], in_=lov_d[:])

            for b in range(BPC):
                ids_t = idxp.tile([128, NCHUNK], i32, tag="ids")
                nc.sync.dma_start(out=ids_t[:], in_=ids_d[b])
                at_all = inp.tile([128, NCHUNK * O], f32, tag="attn")
                nc.sync.dma_start(
                    out=at_all[:].rearrange("p (c o) -> p c o", o=O),
                    in_=attn_d[b].rearrange("(c p) o -> p c o", p=128),
                )

                # hi = ids // 1250 via RTN int cast of (ids+0.5)/1250 - 0.5
                # (exact for [0, 50000), verified on HW); lo = ids - 1250*hi
                ids_f = idxp.tile([128, NCHUNK], f32, tag="idsf")
                nc.vector.tensor_copy(out=ids_f[:], in_=ids_t[:])
                tq = idxp.tile([128, NCHUNK], f32, tag="tq")
                nc.vector.tensor_scalar(out=tq[:], in0=ids_f[:], scalar1=0.5,
                                        scalar2=float(np.float32(1.0 / V2)),
                                        op0=Alu.add, op1=Alu.mult)
                hi_i = idxp.tile([128, NCHUNK], i32, tag="hi")
                nc.vector.tensor_scalar(out=hi_i[:], in0=tq[:], scalar1=0.5,
                                        scalar2=None, op0=Alu.subtract)
                hi_f = idxp.tile([128, NCHUNK], f32, tag="hif")
                nc.vector.tensor_copy(out=hi_f[:], in_=hi_i[:])
                lo_i = idxp.tile([128, NCHUNK], i32, tag="lo")
                nc.vector.scalar_tensor_tensor(out=lo_i[:], in0=hi_i[:],
                                               scalar=-V2, in1=ids_t[:],
                                               op0=Alu.mult, op1=Alu.add)
                lo_f = idxp.tile([128, NCHUNK], f32, tag="lof")
                nc.vector.tensor_copy(out=lo_f[:], in_=lo_i[:])

                gt = gtp.tile([128, NCHUNK * PAIRS], mmdt, tag="gt")
                alo = gtp.tile([128, NCHUNK * V2], mmdt, tag="alo")
                for c in range(NCHUNK):
                    at = at_all[:, c * O : (c + 1) * O]
                    nc.vector.tensor_scalar(out=alo[:, c * V2 : (c + 1) * V2],
                                            in0=lov[:],
                                            scalar1=lo_f[:, c : c + 1],
                                            scalar2=None, op0=Alu.is_equal)
                    bh = idxp.tile([128, PAIRS], mmdt, tag="bh")
                    nc.vector.tensor_scalar(out=bh[:], in0=gidx[:],
                                            scalar1=hi_f[:, c : c + 1],
                                            scalar2=None, op0=Alu.is_equal)
                    # split gt by pair-halves so early tiles' matmuls unblock
                    # before the full chunk is built
                    for p0, p1 in ((0, 16), (16, O)):
                        nc.vector.tensor_tensor(
                            out=gt[:, c * PAIRS + p0 * V1 : c * PAIRS + p1 * V1]
                            .rearrange("p (o g) -> p o g", g=V1),
                            in0=at[:, p0:p1].unsqueeze(2).broadcast_to(
                                [128, p1 - p0, V1]),
                            in1=bh[:, p0 * V1 : p1 * V1].rearrange(
                                "p (o g) -> p o g", g=V1),
                            op=Alu.mult,
                        )

                out_flat = out_d[b].rearrange("o v -> (o v)")
                for t in range(NTILES):
                    w = 128
                    ps = psmm.tile([128, V2], f32, tag="mm")
                    for c in range(NCHUNK):
                        for n0, n1 in NSLICES:
                            nc.tensor.matmul(
                                out=ps[:w, n0:n1],
                                lhsT=gt[:, c * PAIRS + t * 128 : c * PAIRS + t * 128 + w],
                                rhs=alo[:, c * V2 + n0 : c * V2 + n1],
                                start=(c == 0),
                                stop=(c == NCHUNK - 1),
                            )
                    os_ = outp.tile([128, V2], f32, tag="os")
                    last = b == BPC - 1 and t == NTILES - 1
                    halves = ((0, V2 // 2), (V2 // 2, V2)) if last else ((0, V2),)
                    for k, (v0, v1) in enumerate(halves):
                        if (t + k) % 2 == 0:
                            nc.scalar.copy(out=os_[:w, v0:v1], in_=ps[:w, v0:v1])
                        else:
                            nc.vector.tensor_copy(out=os_[:w, v0:v1],
                                                  in_=ps[:w, v0:v1])
                        nc.sync.dma_start(
                            out=out_flat[t * 128 * V2 : t * 128 * V2 + w * V2]
                            .rearrange("(p l) -> p l", l=V2)[:, v0:v1],
                            in_=os_[:w, v0:v1],
                        )

    nc.compile()
    return nc


def _consts():
    gidx = np.broadcast_to(
        np.tile(np.arange(V1, dtype=np.float16), O)[None, :], (128, PAIRS)
    ).copy()
    lov = np.broadcast_to(
        np.arange(V2, dtype=np.float16)[None, :], (128, V2)
    ).copy()
    return gidx, lov


def kernel(ids, attn):
    from concourse.bass_utils import run_bass_kernel_spmd

    ids = np.ascontiguousarray(ids, dtype=np.int32)
    attn = np.ascontiguousarray(attn, dtype=np.float32)

    if "nc" not in _cache:
        _cache["nc"] = _build()
    nc = _cache["nc"]

    gidx, lov = _consts()
    # [B, I] -> per core [BPC, 128, NCHUNK] with [b, p, c] = ids[b, c*128 + p]
    ids_t = ids.reshape(B, NCHUNK, 128).transpose(0, 2, 1)
    attn_t = attn.transpose(0, 2, 1)  # [B, I, O]
    core_ids = list(range(NCORES))
    in_maps = [
        {
            "ids": np.ascontiguousarray(ids_t[c * BPC : (c + 1) * BPC]),
            "attn": np.ascontiguousarray(attn_t[c * BPC : (c + 1) * BPC]),
            "gidx": gidx,
            "lov": lov,
        }
        for c in core_ids
    ]
    res = run_bass_kernel_spmd(nc, in_maps, core_ids)
    out = np.concatenate([res.results[c]["out"] for c in core_ids], axis=0)
    return out


# revision 9
# speedup vs baseline: 2.1867x; 1.0031x over previous
"""Trainium2 Bass kernel for batched weighted scatter-add (AttentionCopy).

Computes out[b, o, v] = sum_i attn[b, o, i] * (ids[b, i] == v)
for ids [16, 512] int32 in [0, 50000), attn [16, 32, 512] f32,
out [16, 32, 50000] f32.

Strategy: pure data parallel over the batch dim — 2 batches per core on 8
cores. Per batch the output is built *densely* on-device via a two-level
one-hot factorization of the vocab index (v = g*2000 + lo, 25 groups):

  out[o, g, lo] = sum_i (attnT[i, o] * (hi_i == g)) * (lo_i == lo)
                = (G_(o,g)).T @ Alo      (PE matmul, contraction over i)

The masked matrix G.T[i, (o, g)] and the one-hot Alo[i, lo] are built with
iota-constant compares on the vector engine. With o-major (o, g) pairs and
V2 = 1250 (40 * 1250 = 50000 exactly), each [128, 1250] f32 PSUM tile maps
to a *flat contiguous* 640 KB run of the row-major [32, 50000] output, so
every DRAM write is a full-partition coalesced DMA (all 16 SDMA engines),
and the 1280 pairs split into exactly 10 full 128-partition tiles.

hi = ids // 1250 uses the round-to-nearest int cast of
(ids + 0.5) * (1/1250) - 0.5, validated exhaustively on HW for [0, 50000).
"""

import sys

sys.path.insert(0, "/opt/trn_rl_repo")

import numpy as np

NCORES = 8
B, O, I = 16, 32, 512
SIZE = 50000
BPC = B // NCORES  # batches per core
V2 = 1250  # lo range (3 PSUM banks per tile)
V1 = 40  # number of vocab groups: 40 * 1250 = 50000 exactly
PAIRS = O * V1  # 1280 (o, g) pairs per batch, o-major
NTILES = PAIRS // 128  # exactly 10 full 128-pair tiles
NCHUNK = I // 128  # 4 contraction chunks
# matmul N-slices of V2, each within one 2 KiB PSUM bank
NSLICES = [(0, 512), (512, 1024), (1024, 1250)]

_cache = {}


def _build(mm_dtype="bfloat16"):
    import concourse.bacc as bacc
    import concourse.mybir as mybir
    import concourse.tile as tile

    f32 = mybir.dt.float32
    f16 = mybir.dt.float16
    mmdt = getattr(mybir.dt, mm_dtype)
    i32 = mybir.dt.int32
    Alu = mybir.AluOpType

    nc = bacc.Bacc("TRN2", target_bir_lowering=False, debug=False, num_devices=NCORES)

    # ids pre-tiled on host to [BPC, 128, NCHUNK] with [p, c] = ids[c*128 + p]
    ids_d = nc.dram_tensor("ids", [BPC, 128, NCHUNK], i32, kind="ExternalInput").ap()
    attn_d = nc.dram_tensor("attn", [BPC, I, O], f32, kind="ExternalInput").ap()
    gidx_d = nc.dram_tensor("gidx", [128, PAIRS], f16, kind="ExternalInput").ap()
    lov_d = nc.dram_tensor("lov", [128, V2], f16, kind="ExternalInput").ap()
    out_d = nc.dram_tensor("out", [BPC, O, SIZE], f32, kind="ExternalOutput").ap()

    with tile.TileContext(nc) as tc:
        with (
            tc.tile_pool(name="const", bufs=1) as constp,
            tc.tile_pool(name="inp", bufs=2) as inp,
            tc.tile_pool(name="idx", bufs=2) as idxp,
            tc.tile_pool(name="gt", bufs=2) as gtp,
            tc.tile_pool(name="outs", bufs=4) as outp,
            tc.tile_pool(name="psmm", bufs=2, space="PSUM") as psmm,
        ):
            warm = constp.tile([128, 256], mmdt)
            nc.gpsimd.memset(warm[:], 0)
            wps = psmm.tile([128, 256], f32, tag="mm")
            for _ in range(36):
                nc.tensor.matmul(out=wps[:, :256], lhsT=warm[:, :128],
                                 rhs=warm[:, :256], start=True, stop=True)
            gidx = constp.tile([128, PAIRS], f16)
            nc.sync.dma_start(out=gidx[:], in_=gidx_d[:])
            lov = constp.tile([128, V2], f16)
            nc.sync.dma_start(out=lov[:], in_=lov_d[:])

            for b in range(BPC):
                ids_t = idxp.tile([128, NCHUNK], i32, tag="ids")
                nc.sync.dma_start(out=ids_t[:], in_=ids_d[b])
                at_all = inp.tile([128, NCHUNK * O], f32, tag="attn")
                nc.sync.dma_start(
                    out=at_all[:].rearrange("p (c o) -> p c o", o=O),
                    in_=attn_d[b].rearrange("(c p) o -> p c o", p=128),
                )

                # hi = ids // 1250 via RTN int cast of (ids+0.5)/1250 - 0.5
                # (exact for [0, 50000), verified on HW); lo = ids - 1250*hi
                ids_f = idxp.tile([128, NCHUNK], f32, tag="idsf")
                nc.vector.tensor_copy(out=ids_f[:], in_=ids_t[:])
                tq = idxp.tile([128, NCHUNK], f32, tag="tq")
                nc.vector.tensor_scalar(out=tq[:], in0=ids_f[:], scalar1=0.5,
                                        scalar2=float(np.float32(1.0 / V2)),
                                        op0=Alu.add, op1=Alu.mult)
                hi_i = idxp.tile([128, NCHUNK], i32, tag="hi")
                nc.vector.tensor_scalar(out=hi_i[:], in0=tq[:], scalar1=0.5,
                                        scalar2=None, op0=Alu.subtract)
                hi_f = idxp.tile([128, NCHUNK], f32, tag="hif")
                nc.vector.tensor_copy(out=hi_f[:], in_=hi_i[:])
                lo_i = idxp.tile([128, NCHUNK], i32, tag="lo")
                nc.vector.scalar_tensor_tensor(out=lo_i[:], in0=hi_i[:],
                                               scalar=-V2, in1=ids_t[:],
                                               op0=Alu.mult, op1=Alu.add)
                lo_f = idxp.tile([128, NCHUNK], f32, tag="lof")
                nc.vector.tensor_copy(out=lo_f[:], in_=lo_i[:])

                gt = gtp.tile([128, NCHUNK * PAIRS], mmdt, tag="gt")
                alo = gtp.tile([128, NCHUNK * V2], mmdt, tag="alo")
                for c in range(NCHUNK):
                    at = at_all[:, c * O : (c + 1) * O]
                    nc.vector.tensor_scalar(out=alo[:, c * V2 : (c + 1) * V2],
                                            in0=lov[:],
                                            scalar1=lo_f[:, c : c + 1],
                                            scalar2=None, op0=Alu.is_equal)
                    bh = idxp.tile([128, PAIRS], mmdt, tag="bh")
                    nc.vector.tensor_scalar(out=bh[:], in0=gidx[:],
                                            scalar1=hi_f[:, c : c + 1],
                                            scalar2=None, op0=Alu.is_equal)
                    # split gt by pair-halves so early tiles' matmuls unblock
                    # before the full chunk is built
                    for p0, p1 in ((0, 16), (16, O)):
                        nc.vector.tensor_tensor(
                            out=gt[:, c * PAIRS + p0 * V1 : c * PAIRS + p1 * V1]
                            .rearrange("p (o g) -> p o g", g=V1),
                            in0=at[:, p0:p1].unsqueeze(2).broadcast_to(
                                [128, p1 - p0, V1]),
                            in1=bh[:, p0 * V1 : p1 * V1].rearrange(
                                "p (o g) -> p o g", g=V1),
                            op=Alu.mult,
                        )

                out_flat = out_d[b].rearrange("o v -> (o v)")
                for t in range(NTILES):
                    w = 128
                    ps = psmm.tile([128, V2], f32, tag="mm")
                    for c in range(NCHUNK):
                        for n0, n1 in NSLICES:
                            nc.tensor.matmul(
                                out=ps[:w, n0:n1],
                                lhsT=gt[:, c * PAIRS + t * 128 : c * PAIRS + t * 128 + w],
                                rhs=alo[:, c * V2 + n0 : c * V2 + n1],
                                start=(c == 0),
                                stop=(c == NCHUNK - 1),
                            )
                    os_ = outp.tile([128, V2], f32, tag="os")
                    last = b == BPC - 1 and t == NTILES - 1
                    halves = ((0, V2 // 2), (V2 // 2, V2)) if last else ((0, V2),)
                    for k, (v0, v1) in enumerate(halves):
                        nc.scalar.copy(out=os_[:w, v0:v1], in_=ps[:w, v0:v1])
                        nc.sync.dma_start(
                            out=out_flat[t * 128 * V2 : t * 128 * V2 + w * V2]
                            .rearrange("(p l) -> p l", l=V2)[:, v0:v1],
                            in_=os_[:w, v0:v1],
                        )

    nc.compile()
    return nc


def _consts():
    gidx = np.broadcast_to(
        np.tile(np.arange(V1, dtype=np.float16), O)[None, :], (128, PAIRS)
    ).copy()
    lov = np.broadcast_to(
        np.arange(V2, dtype=np.float16)[None, :], (128, V2)
    ).copy()
    return gidx, lov


def kernel(ids, attn):
    from concourse.bass_utils import run_bass_kernel_spmd

    ids = np.ascontiguousarray(ids, dtype=np.int32)
    attn = np.ascontiguousarray(attn, dtype=np.float32)

    if "nc" not in _cache:
        _cache["nc"] = _build()
    nc = _cache["nc"]

    gidx, lov = _consts()
    # [B, I] -> per core [BPC, 128, NCHUNK] with [b, p, c] = ids[b, c*128 + p]
    ids_t = ids.reshape(B, NCHUNK, 128).transpose(0, 2, 1)
    attn_t = attn.transpose(0, 2, 1)  # [B, I, O]
    core_ids = list(range(NCORES))
    in_maps = [
        {
            "ids": np.ascontiguousarray(ids_t[c * BPC : (c + 1) * BPC]),
            "attn": np.ascontiguousarray(attn_t[c * BPC : (c + 1) * BPC]),
            "gidx": gidx,
            "lov": lov,
        }
        for c in core_ids
    ]
    res = run_bass_kernel_spmd(nc, in_maps, core_ids)
    out = np.concatenate([res.results[c]["out"] for c in core_ids], axis=0)
    return out


# revision 11
# speedup vs baseline: 2.3681x; 1.0830x over previous
"""Trainium2 Bass kernel for batched weighted scatter-add (AttentionCopy).

Computes out[b, o, v] = sum_i attn[b, o, i] * (ids[b, i] == v)
for ids [16, 512] int32 in [0, 50000), attn [16, 32, 512] f32,
out [16, 32, 50000] f32.

Strategy: pure data parallel over the batch dim — 2 batches per core on 8
cores. Per batch the output is built *densely* on-device via a two-level
one-hot factorization of the vocab index (v = g*2000 + lo, 25 groups):

  out[o, g, lo] = sum_i (attnT[i, o] * (hi_i == g)) * (lo_i == lo)
                = (G_(o,g)).T @ Alo      (PE matmul, contraction over i)

The masked matrix G.T[i, (o, g)] and the one-hot Alo[i, lo] are built with
iota-constant compares on the vector engine. With o-major (o, g) pairs and
V2 = 1250 (40 * 1250 = 50000 exactly), each [128, 1250] f32 PSUM tile maps
to a *flat contiguous* 640 KB run of the row-major [32, 50000] output, so
every DRAM write is a full-partition coalesced DMA (all 16 SDMA engines),
and the 1280 pairs split into exactly 10 full 128-partition tiles.

hi = ids // 1250 uses the round-to-nearest int cast of
(ids + 0.5) * (1/1250) - 0.5, validated exhaustively on HW for [0, 50000).
"""

import sys

sys.path.insert(0, "/opt/trn_rl_repo")

import numpy as np

NCORES = 8
B, O, I = 16, 32, 512
SIZE = 50000
BPC = B // NCORES  # batches per core
V2 = 1250  # lo range (3 PSUM banks per tile)
V1 = 40  # number of vocab groups: 40 * 1250 = 50000 exactly
PAIRS = O * V1  # 1280 (o, g) pairs per batch, o-major
NTILES = PAIRS // 128  # exactly 10 full 128-pair tiles
NCHUNK = I // 128  # 4 contraction chunks
# matmul N-slices of V2, each within one 2 KiB PSUM bank
NSLICES = [(0, 512), (512, 1024), (1024, 1250)]

_cache = {}


def _build(mm_dtype="bfloat16"):
    import concourse.bacc as bacc
    import concourse.mybir as mybir
    import concourse.tile as tile

    f32 = mybir.dt.float32
    f16 = mybir.dt.float16
    mmdt = getattr(mybir.dt, mm_dtype)
    i32 = mybir.dt.int32
    Alu = mybir.AluOpType

    nc = bacc.Bacc("TRN2", target_bir_lowering=False, debug=False, num_devices=NCORES)

    # ids pre-tiled on host to [BPC, 128, NCHUNK] with [p, c] = ids[c*128 + p]
    ids_d = nc.dram_tensor("ids", [BPC, 128, NCHUNK], i32, kind="ExternalInput").ap()
    attn_d = nc.dram_tensor("attn", [BPC, I, O], f32, kind="ExternalInput").ap()
    gidx_d = nc.dram_tensor("gidx", [128, PAIRS], f16, kind="ExternalInput").ap()
    lov_d = nc.dram_tensor("lov", [128, V2], f16, kind="ExternalInput").ap()
    out_d = nc.dram_tensor("out", [BPC, O, SIZE], f32, kind="ExternalOutput").ap()

    with tile.TileContext(nc) as tc:
        with (
            tc.tile_pool(name="const", bufs=1) as constp,
            tc.tile_pool(name="inp", bufs=2) as inp,
            tc.tile_pool(name="idx", bufs=2) as idxp,
            tc.tile_pool(name="gt", bufs=2) as gtp,
            tc.tile_pool(name="outs", bufs=4) as outp,
            tc.tile_pool(name="psmm", bufs=2, space="PSUM") as psmm,
        ):
            warm = constp.tile([128, 256], mmdt)
            nc.gpsimd.memset(warm[:], 0)
            wps = psmm.tile([128, 256], f32, tag="mm")
            for _ in range(40):
                nc.tensor.matmul(out=wps[:, :256], lhsT=warm[:, :128],
                                 rhs=warm[:, :256], start=True, stop=True)
            ids_ts, at_alls = [], []
            for b in range(BPC):
                ids_ts.append(idxp.tile([128, NCHUNK], i32, tag=f"ids{b}", name=f"ids_t{b}"))
                at_alls.append(inp.tile([128, NCHUNK * O], f32, tag=f"attn{b}", name=f"at_all{b}"))
            nc.sync.dma_start(out=ids_ts[0][:], in_=ids_d[0])
            lov = constp.tile([128, V2], f16)
            nc.sync.dma_start(out=lov[:], in_=lov_d[:])
            gidx = constp.tile([128, PAIRS], f16)
            nc.sync.dma_start(out=gidx[:], in_=gidx_d[:])
            for b in range(BPC):
                if b > 0:
                    nc.sync.dma_start(out=ids_ts[b][:], in_=ids_d[b])
                nc.sync.dma_start(
                    out=at_alls[b][:].rearrange("p (c o) -> p c o", o=O),
                    in_=attn_d[b].rearrange("(c p) o -> p c o", p=128),
                )

            for b in range(BPC):
                ids_t = ids_ts[b]
                at_all = at_alls[b]

                # hi = ids // 1250 via RTN int cast of (ids+0.5)/1250 - 0.5
                # (exact for [0, 50000), verified on HW); lo = ids - 1250*hi
                ids_f = idxp.tile([128, NCHUNK], f32, tag="idsf")
                nc.vector.tensor_copy(out=ids_f[:], in_=ids_t[:])
                tq = idxp.tile([128, NCHUNK], f32, tag="tq")
                nc.vector.tensor_scalar(out=tq[:], in0=ids_f[:], scalar1=0.5,
                                        scalar2=float(np.float32(1.0 / V2)),
                                        op0=Alu.add, op1=Alu.mult)
                hi_i = idxp.tile([128, NCHUNK], i32, tag="hi")
                nc.vector.tensor_scalar(out=hi_i[:], in0=tq[:], scalar1=0.5,
                                        scalar2=None, op0=Alu.subtract)
                hi_f = idxp.tile([128, NCHUNK], f32, tag="hif")
                nc.vector.tensor_copy(out=hi_f[:], in_=hi_i[:])
                lo_i = idxp.tile([128, NCHUNK], i32, tag="lo")
                nc.vector.scalar_tensor_tensor(out=lo_i[:], in0=hi_i[:],
                                               scalar=-V2, in1=ids_t[:],
                                               op0=Alu.mult, op1=Alu.add)
                lo_f = idxp.tile([128, NCHUNK], f32, tag="lof")
                nc.vector.tensor_copy(out=lo_f[:], in_=lo_i[:])

                gt = gtp.tile([128, NCHUNK * PAIRS], mmdt, tag="gt")
                alo = gtp.tile([128, NCHUNK * V2], mmdt, tag="alo")
                bhs = []
                for c in range(NCHUNK):
                    nc.vector.tensor_scalar(out=alo[:, c * V2 : (c + 1) * V2],
                                            in0=lov[:],
                                            scalar1=lo_f[:, c : c + 1],
                                            scalar2=None, op0=Alu.is_equal)
                    bh = idxp.tile([128, PAIRS], mmdt, tag=f"bh{c}")
                    nc.vector.tensor_scalar(out=bh[:], in0=gidx[:],
                                            scalar1=hi_f[:, c : c + 1],
                                            scalar2=None, op0=Alu.is_equal)
                    bhs.append(bh)
                # gt split by pair-halves; emit all first halves before the
                # second halves so tiles 0-4 unblock as early as possible
                for p0, p1 in ((0, 16), (16, O)):
                    for c in range(NCHUNK):
                        at = at_all[:, c * O : (c + 1) * O]
                        nc.vector.tensor_tensor(
                            out=gt[:, c * PAIRS + p0 * V1 : c * PAIRS + p1 * V1]
                            .rearrange("p (o g) -> p o g", g=V1),
                            in0=at[:, p0:p1].unsqueeze(2).broadcast_to(
                                [128, p1 - p0, V1]),
                            in1=bhs[c][:, p0 * V1 : p1 * V1].rearrange(
                                "p (o g) -> p o g", g=V1),
                            op=Alu.mult,
                        )

                out_flat = out_d[b].rearrange("o v -> (o v)")
                for t in range(NTILES):
                    w = 128
                    ps = psmm.tile([128, V2], f32, tag="mm")
                    for c in range(NCHUNK):
                        for n0, n1 in NSLICES:
                            nc.tensor.matmul(
                                out=ps[:w, n0:n1],
                                lhsT=gt[:, c * PAIRS + t * 128 : c * PAIRS + t * 128 + w],
                                rhs=alo[:, c * V2 + n0 : c * V2 + n1],
                                start=(c == 0),
                                stop=(c == NCHUNK - 1),
                            )
                    os_ = outp.tile([128, V2], f32, tag="os")
                    last = b == BPC - 1 and t == NTILES - 1
                    halves = ((0, V2 // 2), (V2 // 2, V2)) if last else ((0, V2),)
                    for k, (v0, v1) in enumerate(halves):
                        nc.scalar.copy(out=os_[:w, v0:v1], in_=ps[:w, v0:v1])
                        nc.sync.dma_start(
                            out=out_flat[t * 128 * V2 : t * 128 * V2 + w * V2]
                            .rearrange("(p l) -> p l", l=V2)[:, v0:v1],
                            in_=os_[:w, v0:v1],
                        )

    nc.compile()
    return nc


def _consts():
    gidx = np.broadcast_to(
        np.tile(np.arange(V1, dtype=np.float16), O)[None, :], (128, PAIRS)
    ).copy()
    lov = np.broadcast_to(
        np.arange(V2, dtype=np.float16)[None, :], (128, V2)
    ).copy()
    return gidx, lov


def kernel(ids, attn):
    from concourse.bass_utils import run_bass_kernel_spmd

    ids = np.ascontiguousarray(ids, dtype=np.int32)
    attn = np.ascontiguousarray(attn, dtype=np.float32)

    if "nc" not in _cache:
        _cache["nc"] = _build()
    nc = _cache["nc"]

    gidx, lov = _consts()
    # [B, I] -> per core [BPC, 128, NCHUNK] with [b, p, c] = ids[b, c*128 + p]
    ids_t = ids.reshape(B, NCHUNK, 128).transpose(0, 2, 1)
    attn_t = attn.transpose(0, 2, 1)  # [B, I, O]
    core_ids = list(range(NCORES))
    in_maps = [
        {
            "ids": np.ascontiguousarray(ids_t[c * BPC : (c + 1) * BPC]),
            "attn": np.ascontiguousarray(attn_t[c * BPC : (c + 1) * BPC]),
            "gidx": gidx,
            "lov": lov,
        }
        for c in core_ids
    ]
    res = run_bass_kernel_spmd(nc, in_maps, core_ids)
    out = np.concatenate([res.results[c]["out"] for c in core_ids], axis=0)
    return out
